# revision 1
# baseline (speedup 1.0000x reference)
"""Bass/Trainium2 kernel for nn_BeMultiHeadAttention (B=2, S=2048, D=1024, H=16, HD=64).

Sharding: data-parallel over tokens. 8 cores; core c handles batch b=c//4 and
query slice q0=(c%4)*512 .. +512. Each core computes K/V projections for its
full batch (2048 keys), Q projection for its 512 queries, transposed-scores
flash attention (no max subtraction needed: |score/8| <~ 2), and the output
projection for its 512 tokens. No collectives; the host concatenates shards.

Layout notes:
 - Everything that needs the contraction dim on partitions is fed from a
   host-pretransposed xT (d-major). Weights are host-packed blockdiagonal per
   head-pair so projections contract over the full 128 partitions.
 - scoresT orientation ([keys, q]) keeps exp output directly consumable as the
   moving operand of the attn@V matmul; per-query softmax sums come for free
   from a ones column appended to V (M=65 matmul).
 - Normalization: recip of the sums row, rank-1 fp32 matmul broadcast
   (ones x recip), DVE multiply + per-partition bias. Odd heads are shifted to
   partitions 64..127 with an identity matmul so the pair tile matches Wo rows.
 - Output bias via a K=1 matmul (ones row x bo) appended to the accumulation.
"""

import numpy as np
import ml_dtypes

import concourse.bass as bass
import concourse.tile as tile
import concourse.mybir as mybir
from concourse.bass_utils import run_bass_kernel_spmd

BF16 = ml_dtypes.bfloat16

B, S, D, H, HD = 2, 2048, 1024, 16, 64
NCORES = 8
QS = S * B // NCORES          # 512 queries per core
NPAIR = H // 2                # 8 head pairs
NKC = S // 128                # 16 key chunks
SCALE = 1.0 / np.sqrt(HD)     # 0.125
GROUP = 2                     # key chunks per exp group
NG = NKC // GROUP             # 8 score groups per head

_bf = mybir.dt.bfloat16
_f32 = mybir.dt.float32


def _split_excess_waits(nc, max_waits=1):
    """This container's walrus only accepts one sync-wait per instruction;
    split extras onto preceding NoOps on the same engine."""
    for fn in nc.m.functions:
        for bb in fn.blocks:
            new_insts = []
            for inst in bb.instructions:
                si = inst.sync_info
                if si is not None and si.on_wait and len(si.on_wait) > max_waits:
                    waits = list(si.on_wait)
                    extra, keep = waits[:-max_waits], waits[-max_waits:]
                    while extra:
                        chunk, extra = extra[:max_waits], extra[max_waits:]
                        new_insts.append(mybir.InstNoOp(
                            name=nc.get_next_instruction_name(),
                            engine=inst.engine,
                            sync_info=mybir.SyncInfo(on_wait=chunk, on_update=[]),
                            bass_nofuse=True))
                    inst.sync_info = mybir.SyncInfo(
                        on_wait=keep, on_update=list(si.on_update))
                new_insts.append(inst)
            bb.instructions = new_insts


def build_nc():
    nc = bass.Bass("TRN2", target_bir_lowering=False, debug=False)

    xt_in = nc.declare_dram_parameter("xt", [128, 8, S], _bf, isOutput=False)
    xtq_in = nc.declare_dram_parameter("xtq", [128, 8, QS], _bf, isOutput=False)
    wk_in = nc.declare_dram_parameter("wk", [128, NPAIR * 128], _bf, isOutput=False)
    wq_in = nc.declare_dram_parameter("wq", [128, NPAIR * 128], _bf, isOutput=False)
    wv_in = nc.declare_dram_parameter("wv", [128, NPAIR * 128], _bf, isOutput=False)
    bk_in = nc.declare_dram_parameter("bk", [128, NPAIR], _f32, isOutput=False)
    bq_in = nc.declare_dram_parameter("bq", [128, NPAIR], _f32, isOutput=False)
    bv_in = nc.declare_dram_parameter("bv", [64, H], _f32, isOutput=False)
    wo_in = nc.declare_dram_parameter("wo", [128, 8, D], _bf, isOutput=False)
    bo_in = nc.declare_dram_parameter("bo", [1, D], _bf, isOutput=False)
    id_in = nc.declare_dram_parameter("ident", [64, 64], _bf, isOutput=False)
    out_d = nc.declare_dram_parameter("out", [QS, D], _f32, isOutput=True)

    Exp = mybir.ActivationFunctionType.Exp
    Log = mybir.ActivationFunctionType.Ln

    with tile.TileContext(nc) as tc:
        with (
            tc.tile_pool(name="singles", bufs=1) as singles,
            tc.tile_pool(name="attn", bufs=3) as attn_pool,
            tc.tile_pool(name="ep", bufs=2) as ep_pool,
            tc.tile_pool(name="ysb", bufs=2) as y_pool,
        ):
            ones_bf = singles.tile([1, 128], _bf)
            nc.vector.memset(ones_bf[:], 1.0)
            warm_rhs = singles.tile([1, 512], _bf)
            nc.vector.memset(warm_rhs[:], 1.0)
            ones_bf2 = singles.tile([128, 64], _bf)
            nc.vector.memset(ones_bf2[:], 1.0)

            # Input DMAs: weights first so pair-0 projections unblock early.
            wk_sb = singles.tile([128, NPAIR * 128], _bf)
            nc.sync.dma_start(wk_sb[:], wk_in[:])
            wq_sb = singles.tile([128, NPAIR * 128], _bf)
            nc.sync.dma_start(wq_sb[:], wq_in[:])
            wv_sb = singles.tile([128, NPAIR * 128], _bf)
            nc.sync.dma_start(wv_sb[:], wv_in[:])
            bk_sb = singles.tile([128, NPAIR], _f32)
            nc.sync.dma_start(bk_sb[:], bk_in[:])
            bq_sb = singles.tile([128, NPAIR], _f32)
            nc.sync.dma_start(bq_sb[:], bq_in[:])
            bv_sb = singles.tile([64, H], _f32)
            nc.sync.dma_start(bv_sb[:], bv_in[:])
            bo_sb = singles.tile([1, D], _bf)
            nc.sync.dma_start(bo_sb[:], bo_in[:])
            id_sb = singles.tile([64, 64], _bf)
            nc.sync.dma_start(id_sb[:], id_in[:])
            xtq_sb = singles.tile([128, 8, QS], _bf)
            nc.sync.dma_start(xtq_sb[:], xtq_in[:])
            xt_sb = singles.tile([128, 8, S], _bf)
            for p in range(NPAIR):
                nc.sync.dma_start(xt_sb[:, p, :], xt_in[:, p, :])
            wo_sb = singles.tile([128, 8, D], _bf)
            nc.sync.dma_start(wo_sb[:], wo_in[:])

            kt_sb = singles.tile([128, NPAIR, S], _bf)
            qt_sb = singles.tile([128, NPAIR, QS], _bf)
            # V layout per (pair, keychunk): [V_A(64) | ones | V_B(64) | ones]
            v_sb = singles.tile([128, NPAIR, NKC, 130], _bf)
            nc.vector.memset(v_sb[:], 1.0)

            otn = [singles.tile([128, QS], _bf, name=f"otn{p}") for p in range(NPAIR)]

            # One shared PSUM slot pool ([128,1024] slots) + OT accumulators.
            with (
                tc.tile_pool(name="pslot", bufs=3, space="PSUM") as slot_pool,
                tc.tile_pool(name="pot", bufs=2, space="PSUM") as ot_pool,
            ):
                def slot(nm):
                    return slot_pool.tile([128, 1024], _f32, tag="slot", name=nm)

                # PE warm-up: dense dummy matmuls (dep only on memsets) bring
                # the HAM clock gate to K=8/8 while the input DMAs land.
                wps = slot("warm")
                for i in range(28):
                    nc.tensor.matmul(wps[:, 0:512], ones_bf[:], warm_rhs[:],
                                     start=True, stop=True)

                def emit_proj(p):
                    ws = slice(p * 128, (p + 1) * 128)
                    # K^T (+bias via ACT Identity, which is in every table set)
                    for g in range(2):
                        ps = slot(f"kt{p}_{g}")
                        for i in range(2):
                            t0 = g * 1024 + i * 512
                            nc.tensor.matmul(
                                ps[:, i * 512:(i + 1) * 512],
                                wk_sb[:, ws],
                                xt_sb[:, p, t0:t0 + 512],
                                start=True, stop=True)
                        nc.vector.tensor_scalar_add(
                            kt_sb[:, p, g * 1024:(g + 1) * 1024], ps[:],
                            bk_sb[:, p:p + 1])
                    psq = slot(f"qt{p}")
                    nc.tensor.matmul(psq[:, 0:QS], wq_sb[:, ws], xtq_sb[:, p, :],
                                     start=True, stop=True)
                    nc.vector.tensor_scalar_add(
                        qt_sb[:, p, :], psq[:, 0:QS], bq_sb[:, p:p + 1])
                    # V token-major: 16 chunks of 128 tokens, 8 per psum slot
                    for g in range(2):
                        psv = slot(f"v{p}_{g}")
                        psv8 = psv.rearrange("p (c e) -> p c e", e=128)
                        for i in range(8):
                            c = g * 8 + i
                            nc.tensor.matmul(
                                psv8[:, i, :],
                                xt_sb[:, p, c * 128:(c + 1) * 128],
                                wv_sb[:, ws],
                                start=True, stop=True)
                        dst = v_sb[:, p, g * 8:(g + 1) * 8, :].rearrange(
                            "p c (h e) -> p c h e", e=65)[:, :, :, 0:64]
                        src = psv.rearrange("p (c h e) -> p c h e", h=2, e=64)
                        nc.vector.tensor_copy(dst, src)

                def emit_epilogue(p, pots):
                    for a in range(2):
                        h = 2 * p + a
                        pot = pots[a]
                        # 1/sums via exp(-log(sums)) on ACT: Ln and Exp share
                        # the natural_log_exp_and_others table set.
                        lnrow = ep_pool.tile([65, QS], _f32, tag="lnrow")
                        nc.scalar.activation(lnrow[64:65, :], pot[64:65, :], Log)
                        recip = ep_pool.tile([65, QS], _bf, tag="recip")
                        nc.scalar.activation(recip[64:65, :], lnrow[64:65, :],
                                             Exp, scale=-1.0)
                        psb = slot(f"ep{p}_{a}")
                        nc.tensor.matmul(psb[0:64, 0:QS], ones_bf2[64:65, :],
                                         recip[64:65, :], start=True, stop=True,
                                         tile_position=(64, 0))
                        bcast = ep_pool.tile([64, QS], _f32, tag="bcast")
                        nc.vector.tensor_copy(bcast[:], psb[0:64, 0:QS])
                        if a == 0:
                            nc.vector.tensor_mul(otn[p][0:64, :], pot[0:64, :],
                                                 bcast[:])
                            nc.vector.tensor_scalar_add(
                                otn[p][0:64, :], otn[p][0:64, :],
                                bv_sb[:, h:h + 1])
                        else:
                            tmpb = ep_pool.tile([64, QS], _bf, tag="tmpb")
                            nc.vector.tensor_mul(tmpb[:], pot[0:64, :], bcast[:])
                            nc.vector.tensor_scalar_add(
                                tmpb[:], tmpb[:], bv_sb[:, h:h + 1])
                            ps2 = slot(f"sh{p}")
                            nc.tensor.matmul(ps2[64:128, 0:QS], id_sb[:], tmpb[:],
                                             start=True, stop=True,
                                             tile_position=(0, 64))
                            nc.vector.tensor_copy(otn[p][64:128, :],
                                                  ps2[64:128, 0:QS])

                prev_ep = {"p": None, "pots": None}

                def emit_attn(p):
                    # One PSUM slot per key chunk holds BOTH heads side by side
                    # ([scoresT_A | scoresT_B]); one exp covers both. OT matmuls
                    # run one chunk behind so the in-order PE never stalls on
                    # the exp of the chunk it just produced; the previous
                    # pair's epilogue matmuls slot in after chunk 1.
                    pots = [ot_pool.tile([65, QS], _f32, tag="pot",
                                         name=f"pot{p}_{a}") for a in range(2)]
                    ats_q = []

                    def emit_ot(c):
                        at = ats_q.pop(0)
                        for a in range(2):
                            nc.tensor.matmul(
                                pots[a][:],
                                v_sb[:, p, c, 65 * a:65 * a + 65],
                                at[:, a * QS:(a + 1) * QS],
                                start=(c == 0), stop=(c == NKC - 1))

                    for c in range(NKC):
                        pss = slot(f"pss{p}_{c}")
                        for a in range(2):
                            r = slice(64 * a, 64 * a + 64)
                            nc.tensor.matmul(
                                pss[:, a * QS:(a + 1) * QS],
                                kt_sb[r, p, c * 128:(c + 1) * 128],
                                qt_sb[r, p, :],
                                start=True, stop=True)
                        at = attn_pool.tile([128, 2 * QS], _bf, tag="at")
                        nc.scalar.activation(at[:], pss[:], Exp, scale=SCALE)
                        ats_q.append(at)
                        if c == 1 and prev_ep["pots"] is not None:
                            emit_epilogue(prev_ep["p"], prev_ep["pots"])
                        if c >= 1:
                            emit_ot(c - 1)
                    emit_ot(NKC - 1)
                    prev_ep["p"], prev_ep["pots"] = p, pots

                # software pipeline: projections run one pair ahead of attention
                for p in range(NPAIR + 1):
                    if p < NPAIR:
                        emit_proj(p)
                    if p >= 1:
                        emit_attn(p - 1)
                emit_epilogue(prev_ep["p"], prev_ep["pots"])

                # ---------------- output projection ----------------
                for j in range(QS // 128):
                    for dh in range(2):
                        dsl = slice(dh * 512, (dh + 1) * 512)
                        py = slot(f"y{j}_{dh}")
                        for k in range(NPAIR):
                            nc.tensor.matmul(
                                py[:, 0:512],
                                otn[k][:, j * 128:(j + 1) * 128],
                                wo_sb[:, k, dsl],
                                start=(k == 0), stop=False)
                        nc.tensor.matmul(py[:, 0:512], ones_bf[0:1, :],
                                         bo_sb[0:1, dsl],
                                         start=False, stop=True)
                        ysb = y_pool.tile([128, 512], _f32, tag="ysb")
                        nc.vector.tensor_copy(ysb[:], py[:, 0:512])
                        nc.sync.dma_start(
                            out_d[j * 128:(j + 1) * 128, dsl], ysb[:])

    _split_excess_waits(nc, 1)
    return nc


def _blockdiag_pack(w):
    """[H, HD, HD] -> [128, NPAIR*128] blockdiagonal per pair, k-major."""
    out = np.zeros((128, NPAIR * 128), np.float32)
    for p in range(NPAIR):
        out[0:64, p * 128 + 0:p * 128 + 64] = w[2 * p]
        out[64:128, p * 128 + 64:p * 128 + 128] = w[2 * p + 1]
    return out.astype(BF16)


def _bias_pack(b):
    """[H, HD] -> [128, NPAIR] (pair bias along partitions)."""
    out = np.zeros((128, NPAIR), np.float32)
    for p in range(NPAIR):
        out[0:64, p] = b[2 * p]
        out[64:128, p] = b[2 * p + 1]
    return out


def prepare_inputs(X, Wq, bq, Wk, bk, Wv, bv, Wo, bo):
    """Host-side shard + pack. Returns in_maps (one dict per core)."""
    X = np.asarray(X, np.float32)
    common = {
        "wk": _blockdiag_pack(np.asarray(Wk, np.float32)),
        "wq": _blockdiag_pack(np.asarray(Wq, np.float32)),
        "wv": _blockdiag_pack(np.asarray(Wv, np.float32)),
        "bk": _bias_pack(np.asarray(bk, np.float32)),
        "bq": _bias_pack(np.asarray(bq, np.float32)),
        "bv": np.asarray(bv, np.float32).T.copy(),          # [64, 16]
        "wo": np.ascontiguousarray(
            np.asarray(Wo, np.float32).reshape(8, 128, D).transpose(1, 0, 2)
        ).astype(BF16),
        "bo": np.asarray(bo, np.float32).reshape(1, D).astype(BF16),
        "ident": np.eye(64, dtype=np.float32).astype(BF16),
    }
    xts = []
    for b in range(B):
        xt = np.ascontiguousarray(X[b].T)                   # [D, S]
        xts.append(np.ascontiguousarray(
            xt.reshape(8, 128, S).transpose(1, 0, 2)).astype(BF16))
    in_maps = []
    for c in range(NCORES):
        b = c // (NCORES // B)
        q0 = (c % (NCORES // B)) * QS
        m = dict(common)
        m["xt"] = xts[b]
        m["xtq"] = np.ascontiguousarray(xts[b][:, :, q0:q0 + QS])
        in_maps.append(m)
    return in_maps


_NC_CACHE = None


def _get_nc():
    global _NC_CACHE
    if _NC_CACHE is None:
        _NC_CACHE = build_nc()
    return _NC_CACHE


def kernel(X, Wq, bq, Wk, bk, Wv, bv, Wo, bo):
    nc = _get_nc()
    in_maps = prepare_inputs(X, Wq, bq, Wk, bk, Wv, bv, Wo, bo)
    res = run_bass_kernel_spmd(nc, in_maps, core_ids=list(range(NCORES)))
    out = np.empty((B, S, D), np.float32)
    for c in range(NCORES):
        b = c // (NCORES // B)
        q0 = (c % (NCORES // B)) * QS
        out[b, q0:q0 + QS, :] = res.results[c]["out"]
    return out



# revision 8
# speedup vs baseline: 1.2877x; 1.2877x over previous
"""Bass/Trainium2 kernel for nn_BeMultiHeadAttention (B=2, S=2048, D=1024, H=16, HD=64).

Sharding: data-parallel over tokens. 8 cores; core c handles batch b=c//4 and
query slice q0=(c%4)*512 .. +512. Each core computes K/V projections for its
full batch (2048 keys), Q projection for its 512 queries, transposed-scores
flash attention (no max subtraction needed: |score/8| <~ 2), and the output
projection for its 512 tokens. No collectives; the host concatenates shards.

v2 structure: a single global chunk pipeline keeps the ACT (exp) chain dense:
 - per global chunk g (pair p=g//16, key chunk c=g%16): scores matmuls ->
   exp -> OT matmuls lagged by 2 chunks, so the PE never waits on the freshly
   produced exp and pair boundaries don't serialize.
 - projections for pair p+1 are spread across pair p's chunk slack.
 - softmax epilogue is decoupled: raw (unnormalized) attention outputs and the
   per-head sums rows are stashed to SBUF right after each pair's last OT
   (freeing the PSUM pot banks fast); reciprocals are computed in 3 batched
   ln/exp ACTIVATEs over multi-head partition blocks; normalization (one
   rank-16 broadcast matmul + DVE mul + bias add per pair) is scheduled into
   late-pair slack. Head-B partition shift (0:64 -> 64:128) rides a SBUF->SBUF
   DMA instead of an identity matmul.
 - output projection is pipelined at the tail: pairs 0..6 accumulate while the
   last pair's epilogue finishes on ACT/DVE.
"""

import numpy as np
import ml_dtypes

import concourse.bass as bass
import concourse.tile as tile
import concourse.mybir as mybir
from concourse.bass_utils import run_bass_kernel_spmd

BF16 = ml_dtypes.bfloat16

B, S, D, H, HD = 2, 2048, 1024, 16, 64
NCORES = 8
QS = S * B // NCORES          # 512 queries per core
NPAIR = H // 2                # 8 head pairs
NKC = S // 128                # 16 key chunks
SCALE = 1.0 / np.sqrt(HD)     # 0.125

_bf = mybir.dt.bfloat16
_f32 = mybir.dt.float32


def _sum_rows(p):
    """Partition rows for pair p's (headA, headB) sums. Batches must start
    32-aligned for the ACT ln/exp (BIR verifier rule), so pairs 0-3 sit at
    rows 0:8, pairs 4-6 at 32:38, pair 7 at 64:66."""
    if p < 4:
        return 2 * p, 2 * p + 1
    if p < 7:
        return 24 + 2 * p, 25 + 2 * p
    return 64, 65


def _split_excess_waits(nc, max_waits=1):
    """This container's walrus only accepts one sync-wait per instruction;
    split extras onto preceding NoOps on the same engine."""
    for fn in nc.m.functions:
        for bb in fn.blocks:
            new_insts = []
            for inst in bb.instructions:
                si = inst.sync_info
                if si is not None and si.on_wait and len(si.on_wait) > max_waits:
                    waits = list(si.on_wait)
                    extra, keep = waits[:-max_waits], waits[-max_waits:]
                    while extra:
                        chunk, extra = extra[:max_waits], extra[max_waits:]
                        new_insts.append(mybir.InstNoOp(
                            name=nc.get_next_instruction_name(),
                            engine=inst.engine,
                            sync_info=mybir.SyncInfo(on_wait=chunk, on_update=[]),
                            bass_nofuse=True))
                    inst.sync_info = mybir.SyncInfo(
                        on_wait=keep, on_update=list(si.on_update))
                new_insts.append(inst)
            bb.instructions = new_insts


def build_nc():
    nc = bass.Bass("TRN2", target_bir_lowering=False, debug=False)

    xt_in = nc.declare_dram_parameter("xt", [128, 8, S], _bf, isOutput=False)
    xtq_in = nc.declare_dram_parameter("xtq", [128, 8, QS], _bf, isOutput=False)
    wk_in = nc.declare_dram_parameter("wk", [128, NPAIR * 128], _bf, isOutput=False)
    wq_in = nc.declare_dram_parameter("wq", [128, NPAIR * 128], _bf, isOutput=False)
    wv_in = nc.declare_dram_parameter("wv", [128, NPAIR * 128], _bf, isOutput=False)
    bk_in = nc.declare_dram_parameter("bk", [128, NPAIR], _f32, isOutput=False)
    bq_in = nc.declare_dram_parameter("bq", [128, NPAIR], _f32, isOutput=False)
    bv2_in = nc.declare_dram_parameter("bv2", [128, NPAIR], _f32, isOutput=False)
    sel2_in = nc.declare_dram_parameter("sel2", [128, NPAIR * 128], _bf, isOutput=False)
    wo_in = nc.declare_dram_parameter("wo", [128, 8, D], _bf, isOutput=False)
    bo_in = nc.declare_dram_parameter("bo", [1, D], _bf, isOutput=False)
    out_d = nc.declare_dram_parameter("out", [QS, D], _f32, isOutput=True)

    Exp = mybir.ActivationFunctionType.Exp
    Log = mybir.ActivationFunctionType.Ln

    with tile.TileContext(nc) as tc:
        with (
            tc.tile_pool(name="singles", bufs=1) as singles,
            tc.tile_pool(name="attn", bufs=3) as attn_pool,
            tc.tile_pool(name="sumt", bufs=2) as sumt_pool,
            tc.tile_pool(name="ysb", bufs=2) as y_pool,
        ):
            ones_bf = singles.tile([1, 128], _bf)
            nc.vector.memset(ones_bf[:], 1.0)
            warm_rhs = singles.tile([1, 512], _bf)
            nc.vector.memset(warm_rhs[:], 1.0)

            # --- input DMAs, priority-ordered: pair-0 projection inputs first
            wk_sb = singles.tile([128, NPAIR * 128], _bf)
            nc.sync.dma_start(wk_sb[:], wk_in[:])
            xt_sb = singles.tile([128, 8, S], _bf)
            nc.sync.dma_start(xt_sb[:, 0, :], xt_in[:, 0, :])
            wq_sb = singles.tile([128, NPAIR * 128], _bf)
            nc.sync.dma_start(wq_sb[:], wq_in[:])
            xtq_sb = singles.tile([128, 8, QS], _bf)
            nc.sync.dma_start(xtq_sb[:, 0:1, :], xtq_in[:, 0:1, :])
            bk_sb = singles.tile([128, NPAIR], _f32)
            nc.sync.dma_start(bk_sb[:], bk_in[:])
            bq_sb = singles.tile([128, NPAIR], _f32)
            nc.sync.dma_start(bq_sb[:], bq_in[:])
            wv_sb = singles.tile([128, NPAIR * 128], _bf)
            nc.sync.dma_start(wv_sb[:], wv_in[:])
            nc.sync.dma_start(xt_sb[:, 1, :], xt_in[:, 1, :])
            nc.sync.dma_start(xtq_sb[:, 1:8, :], xtq_in[:, 1:8, :])
            for p in range(2, NPAIR):
                nc.sync.dma_start(xt_sb[:, p, :], xt_in[:, p, :])
            bv2_sb = singles.tile([128, NPAIR], _f32)
            nc.sync.dma_start(bv2_sb[:], bv2_in[:])
            sel2_sb = singles.tile([128, NPAIR * 128], _bf)
            nc.sync.dma_start(sel2_sb[:], sel2_in[:])
            bo_sb = singles.tile([1, D], _bf)
            nc.sync.dma_start(bo_sb[:], bo_in[:])
            wo_sb = singles.tile([128, 8, D], _bf)
            nc.sync.dma_start(wo_sb[:], wo_in[:])

            kt_sb = singles.tile([128, NPAIR, S], _bf)
            qt_sb = singles.tile([128, NPAIR, QS], _bf)
            # V layout per (pair, keychunk): [V_A(64) | ones | V_B(64) | ones]
            v_sb = singles.tile([128, NPAIR, NKC, 130], _bf)

            otn = [singles.tile([128, QS], _bf, name=f"otn{p}") for p in range(NPAIR)]
            raw = [singles.tile([128, QS], _bf, name=f"raw{p}") for p in range(NPAIR)]
            sums_sb = singles.tile([66, QS], _f32)
            lnrow_sb = singles.tile([66, QS], _f32)
            recip_sb = singles.tile([66, QS], _bf)
            # the rank-16 bcast matmul reads all 16 partitions; zero-fill so
            # not-yet-written heads can't inject NaNs (0 x NaN = NaN on PE)
            nc.vector.memset(recip_sb[:], 0.0)

            def memset_v(p):
                nc.vector.memset(v_sb[:, p, :, :], 1.0)

            memset_v(0)
            memset_v(1)

            with (
                tc.tile_pool(name="pslot", bufs=3, space="PSUM") as slot_pool,
                tc.tile_pool(name="pot", bufs=1, space="PSUM") as ot_pool,
            ):
                def slot(nm):
                    return slot_pool.tile([128, 1024], _f32, tag="slot", name=nm)

                # PE warm-up: dummy matmuls (dep only on memsets) bring the HAM
                # clock gate toward K=8/8 while the first input DMAs land.
                wps = slot("warm")
                for i in range(10):
                    nc.tensor.matmul(wps[:, 0:512], ones_bf[:], warm_rhs[:],
                                     start=True, stop=True)

                def emit_kt(p, g):
                    ws = slice(p * 128, (p + 1) * 128)
                    ps = slot(f"kt{p}_{g}")
                    for i in range(2):
                        t0 = g * 1024 + i * 512
                        nc.tensor.matmul(
                            ps[:, i * 512:(i + 1) * 512],
                            wk_sb[:, ws],
                            xt_sb[:, p, t0:t0 + 512],
                            start=True, stop=True)
                    nc.vector.tensor_scalar_add(
                        kt_sb[:, p, g * 1024:(g + 1) * 1024], ps[:],
                        bk_sb[:, p:p + 1])

                def emit_qt(p):
                    ws = slice(p * 128, (p + 1) * 128)
                    psq = slot(f"qt{p}")
                    nc.tensor.matmul(psq[:, 0:QS], wq_sb[:, ws], xtq_sb[:, p, :],
                                     start=True, stop=True)
                    nc.vector.tensor_scalar_add(
                        qt_sb[:, p, :], psq[:, 0:QS], bq_sb[:, p:p + 1])

                # V proj group g: 4 token chunks, 4 N=128 matmuls into half a
                # slot, CAST out immediately (keeps the slot hold ~1 chunk).
                def emit_v(p, g):
                    ws = slice(p * 128, (p + 1) * 128)
                    psv = slot(f"v{p}_{g}")
                    psv4 = psv.rearrange("p (c e) -> p c e", e=128)
                    for i in range(4):
                        c = g * 4 + i
                        nc.tensor.matmul(
                            psv4[:, i, :],
                            xt_sb[:, p, c * 128:(c + 1) * 128],
                            wv_sb[:, ws],
                            start=True, stop=True)
                    dst = v_sb[:, p, g * 4:(g + 1) * 4, :].rearrange(
                        "p c (h e) -> p c h e", e=65)[:, :, :, 0:64]
                    src = psv[:, 0:512].rearrange(
                        "p (c h e) -> p c h e", h=2, e=64)
                    nc.vector.tensor_copy(dst, src)

                pots = {}

                def emit_scores_exp(p, c, ats):
                    pss = slot(f"pss{p}_{c}")
                    for a in range(2):
                        r = slice(64 * a, 64 * a + 64)
                        nc.tensor.matmul(
                            pss[:, a * QS:(a + 1) * QS],
                            kt_sb[r, p, c * 128:(c + 1) * 128],
                            qt_sb[r, p, :],
                            start=True, stop=True)
                    at = attn_pool.tile([128, 2 * QS], _bf, tag="at",
                                        name=f"at{p}_{c}")
                    nc.scalar.activation(at[:], pss[:], Exp, scale=SCALE)
                    ats.append(at)

                def emit_ot(p, c, ats):
                    at = ats.pop(0)
                    if c == 0:
                        pots[p] = (
                            ot_pool.tile([65, QS], _f32, tag="potA",
                                         name=f"potA{p}"),
                            ot_pool.tile([65, QS], _f32, tag="potB",
                                         name=f"potB{p}"),
                        )
                    pa, pb = pots[p]
                    nc.tensor.matmul(pa[:], v_sb[:, p, c, 0:65],
                                     at[:, 0:QS],
                                     start=(c == 0), stop=(c == NKC - 1))
                    nc.tensor.matmul(pb[:], v_sb[:, p, c, 65:130],
                                     at[:, QS:2 * QS],
                                     start=(c == 0), stop=(c == NKC - 1))

                def emit_stash(p):
                    pa, pb = pots.pop(p)
                    # raw head-A dims -> raw[p][0:64]; sums row A staged
                    nc.vector.tensor_copy(raw[p][0:64, :], pa[0:64, :])
                    sa = sumt_pool.tile([65, QS], _f32, tag="sumA",
                                        name=f"sumA{p}")
                    nc.vector.tensor_copy(sa[64:65, :], pa[64:65, :])
                    # raw head-B dims -> otn[p][0:64] (borrowed), then DMA-shift
                    # to raw[p][64:128]; sums row B staged
                    nc.vector.tensor_copy(otn[p][0:64, :], pb[0:64, :])
                    sb_ = sumt_pool.tile([65, QS], _f32, tag="sumB",
                                         name=f"sumB{p}")
                    nc.vector.tensor_copy(sb_[64:65, :], pb[64:65, :])
                    nc.sync.dma_start(raw[p][64:128, :], otn[p][0:64, :])
                    ra, rb = _sum_rows(p)
                    nc.sync.dma_start(sums_sb[ra:ra + 1, :], sa[64:65, :])
                    nc.sync.dma_start(sums_sb[rb:rb + 1, :], sb_[64:65, :])

                def emit_recip(h0, h1):
                    # 1/sums via exp(-log(sums)): Ln and Exp share the
                    # natural_log_exp_and_others table set.
                    nc.scalar.activation(lnrow_sb[h0:h1, :], sums_sb[h0:h1, :],
                                         Log)
                    nc.scalar.activation(recip_sb[h0:h1, :], lnrow_sb[h0:h1, :],
                                         Exp, scale=-1.0)

                def emit_normalize(p):
                    nb = slot(f"nb{p}")
                    nc.tensor.matmul(nb[:, 0:QS],
                                     sel2_sb[0:66, p * 128:(p + 1) * 128],
                                     recip_sb[0:66, :], start=True, stop=True)
                    nc.vector.tensor_mul(otn[p][:, :], raw[p][:, :],
                                         nb[:, 0:QS])
                    nc.vector.tensor_scalar_add(otn[p][:, :], otn[p][:, :],
                                                bv2_sb[:, p:p + 1])

                # -------- interleave schedule: (p, c) -> list of thunks ------
                sched = {}

                def at_(p, c, fn, *args):
                    sched.setdefault((p, c), []).append((fn, args))

                for p in range(0, NPAIR - 1):
                    pn = p + 1
                    at_(p, 3, emit_kt, pn, 0)
                    at_(p, 5, emit_kt, pn, 1)
                    at_(p, 7, emit_qt, pn)
                    if pn + 1 < NPAIR:
                        at_(p, 6, memset_v, pn + 1)
                    at_(p, 8, emit_v, pn, 0)
                    at_(p, 10, emit_v, pn, 1)
                    at_(p, 12, emit_v, pn, 2)
                    at_(p, 14, emit_v, pn, 3)
                for p in range(1, NPAIR):
                    at_(p, 1, emit_stash, p - 1)
                at_(5, 6, emit_recip, 0, 8)
                at_(7, 6, emit_recip, 32, 38)
                at_(5, 9, emit_normalize, 0)
                at_(5, 12, emit_normalize, 1)
                at_(6, 2, emit_normalize, 2)
                at_(6, 6, emit_normalize, 3)
                at_(7, 9, emit_normalize, 4)
                at_(7, 11, emit_normalize, 5)
                at_(7, 13, emit_normalize, 6)

                # -------- projections for pair 0 up front --------------------
                emit_kt(0, 0)
                emit_kt(0, 1)
                emit_qt(0)
                for g in range(4):
                    emit_v(0, g)

                # -------- global chunk pipeline ------------------------------
                ats = []
                for g in range(NPAIR * NKC + 2):
                    if g < NPAIR * NKC:
                        p, c = divmod(g, NKC)
                        emit_scores_exp(p, c, ats)
                    if g >= 2:
                        po, co = divmod(g - 2, NKC)
                        emit_ot(po, co, ats)
                    if g < NPAIR * NKC:
                        for fn, args in sched.get((p, c), ()):
                            fn(*args)

                # -------- tail: last pair epilogue + output projection -------
                emit_stash(NPAIR - 1)
                emit_recip(64, 66)

                ybufs = {}

                def emit_y_head(j, dh):
                    dsl = slice(dh * 512, (dh + 1) * 512)
                    py = slot(f"y{j}_{dh}")
                    ybufs[(j, dh)] = py
                    for k in range(NPAIR - 1):
                        nc.tensor.matmul(
                            py[:, 0:512],
                            otn[k][:, j * 128:(j + 1) * 128],
                            wo_sb[:, k, dsl],
                            start=(k == 0), stop=False)

                def emit_y_fin(j, dh):
                    dsl = slice(dh * 512, (dh + 1) * 512)
                    py = ybufs.pop((j, dh))
                    k = NPAIR - 1
                    nc.tensor.matmul(py[:, 0:512],
                                     otn[k][:, j * 128:(j + 1) * 128],
                                     wo_sb[:, k, dsl],
                                     start=False, stop=False)
                    nc.tensor.matmul(py[:, 0:512], ones_bf[0:1, :],
                                     bo_sb[0:1, dsl],
                                     start=False, stop=True)
                    ysb = y_pool.tile([128, 512], _f32, tag="ysb")
                    nc.vector.tensor_copy(ysb[:], py[:, 0:512])
                    nc.sync.dma_start(
                        out_d[j * 128:(j + 1) * 128, dsl], ysb[:])

                # nb7 must be allocated before any y head (slot rotation:
                # keeping <=2 accumulating y banks in flight avoids a
                # PE-order deadlock on bank reuse).
                emit_normalize(NPAIR - 1)
                groups = [(j, dh) for j in range(QS // 128) for dh in range(2)]
                emit_y_head(*groups[0])
                emit_y_head(*groups[1])
                for i in range(2, len(groups)):
                    emit_y_fin(*groups[i - 2])
                    emit_y_head(*groups[i])
                for i in range(len(groups) - 2, len(groups)):
                    emit_y_fin(*groups[i])

    _split_excess_waits(nc, 1)
    return nc


def _blockdiag_pack(w):
    """[H, HD, HD] -> [128, NPAIR*128] blockdiagonal per pair, k-major."""
    out = np.zeros((128, NPAIR * 128), np.float32)
    for p in range(NPAIR):
        out[0:64, p * 128 + 0:p * 128 + 64] = w[2 * p]
        out[64:128, p * 128 + 64:p * 128 + 128] = w[2 * p + 1]
    return out.astype(BF16)


def _bias_pack(b):
    """[H, HD] -> [128, NPAIR] (pair bias along partitions)."""
    out = np.zeros((128, NPAIR), np.float32)
    for p in range(NPAIR):
        out[0:64, p] = b[2 * p]
        out[64:128, p] = b[2 * p + 1]
    return out


def _sel2_pack():
    """[128, NPAIR*128]: pair p block has row _sum_rows(p)[0] = [ones64|0],
    row _sum_rows(p)[1] = [0|ones64] so one rank-66 matmul broadcasts both
    heads' recip rows to the pair's 128 output partitions."""
    out = np.zeros((128, NPAIR * 128), np.float32)
    for p in range(NPAIR):
        ra, rb = _sum_rows(p)
        out[ra, p * 128:p * 128 + 64] = 1.0
        out[rb, p * 128 + 64:p * 128 + 128] = 1.0
    return out.astype(BF16)


def prepare_inputs(X, Wq, bq, Wk, bk, Wv, bv, Wo, bo):
    """Host-side shard + pack. Returns in_maps (one dict per core)."""
    X = np.asarray(X, np.float32)
    common = {
        "wk": _blockdiag_pack(np.asarray(Wk, np.float32)),
        "wq": _blockdiag_pack(np.asarray(Wq, np.float32)),
        "wv": _blockdiag_pack(np.asarray(Wv, np.float32)),
        "bk": _bias_pack(np.asarray(bk, np.float32)),
        "bq": _bias_pack(np.asarray(bq, np.float32)),
        "bv2": _bias_pack(np.asarray(bv, np.float32)),
        "sel2": _sel2_pack(),
        "wo": np.ascontiguousarray(
            np.asarray(Wo, np.float32).reshape(8, 128, D).transpose(1, 0, 2)
        ).astype(BF16),
        "bo": np.asarray(bo, np.float32).reshape(1, D).astype(BF16),
    }
    xts = []
    for b in range(B):
        xt = np.ascontiguousarray(X[b].T)                   # [D, S]
        xts.append(np.ascontiguousarray(
            xt.reshape(8, 128, S).transpose(1, 0, 2)).astype(BF16))
    in_maps = []
    for c in range(NCORES):
        b = c // (NCORES // B)
        q0 = (c % (NCORES // B)) * QS
        m = dict(common)
        m["xt"] = xts[b]
        m["xtq"] = np.ascontiguousarray(xts[b][:, :, q0:q0 + QS])
        in_maps.append(m)
    return in_maps


_NC_CACHE = None


def _get_nc():
    global _NC_CACHE
    if _NC_CACHE is None:
        _NC_CACHE = build_nc()
    return _NC_CACHE


def kernel(X, Wq, bq, Wk, bk, Wv, bv, Wo, bo):
    nc = _get_nc()
    in_maps = prepare_inputs(X, Wq, bq, Wk, bk, Wv, bv, Wo, bo)
    res = run_bass_kernel_spmd(nc, in_maps, core_ids=list(range(NCORES)))
    out = np.empty((B, S, D), np.float32)
    for c in range(NCORES):
        b = c // (NCORES // B)
        q0 = (c % (NCORES // B)) * QS
        out[b, q0:q0 + QS, :] = res.results[c]["out"]
    return out


# revision 10
# speedup vs baseline: 1.3638x; 1.0591x over previous
"""Bass/Trainium2 kernel for nn_BeMultiHeadAttention (B=2, S=2048, D=1024, H=16, HD=64).

Sharding: data-parallel over tokens. 8 cores; core c handles batch b=c//4 and
query slice q0=(c%4)*512 .. +512. Each core computes K/V projections for its
full batch (2048 keys), Q projection for its 512 queries, transposed-scores
flash attention (no max subtraction needed: |score/8| <~ 2), and the output
projection for its 512 tokens. No collectives; the host concatenates shards.

v2 structure: a single global chunk pipeline keeps the ACT (exp) chain dense:
 - per global chunk g (pair p=g//16, key chunk c=g%16): scores matmuls ->
   exp -> OT matmuls lagged by 2 chunks, so the PE never waits on the freshly
   produced exp and pair boundaries don't serialize.
 - projections for pair p+1 are spread across pair p's chunk slack.
 - softmax epilogue is decoupled: raw (unnormalized) attention outputs and the
   per-head sums rows are stashed to SBUF right after each pair's last OT
   (freeing the PSUM pot banks fast); reciprocals are computed in 3 batched
   ln/exp ACTIVATEs over multi-head partition blocks; normalization (one
   rank-16 broadcast matmul + DVE mul + bias add per pair) is scheduled into
   late-pair slack. Head-B partition shift (0:64 -> 64:128) rides a SBUF->SBUF
   DMA instead of an identity matmul.
 - output projection is pipelined at the tail: pairs 0..6 accumulate while the
   last pair's epilogue finishes on ACT/DVE.
"""

import numpy as np
import ml_dtypes

import concourse.bass as bass
import concourse.tile as tile
import concourse.mybir as mybir
from concourse.bass_utils import run_bass_kernel_spmd

BF16 = ml_dtypes.bfloat16

B, S, D, H, HD = 2, 2048, 1024, 16, 64
NCORES = 8
QS = S * B // NCORES          # 512 queries per core
NPAIR = H // 2                # 8 head pairs
NKC = S // 128                # 16 key chunks
SCALE = 1.0 / np.sqrt(HD)     # 0.125

_bf = mybir.dt.bfloat16
_f32 = mybir.dt.float32


def _sum_rows(p):
    """Partition rows for pair p's (headA, headB) sums. ACT ops may only
    start at partition 0/32/64/96 (BIR verifier rule), so the recip batches
    sit at: pairs 0-3 -> rows 0:8, pairs 4-5 -> 32:36, pair 6 -> 96:98,
    pair 7 -> 64:66."""
    if p < 4:
        return 2 * p, 2 * p + 1
    if p < 6:
        return 24 + 2 * p, 25 + 2 * p
    if p == 6:
        return 96, 97
    return 64, 65


def _split_excess_waits(nc, max_waits=1):
    """This container's walrus only accepts one sync-wait per instruction;
    split extras onto preceding NoOps on the same engine."""
    for fn in nc.m.functions:
        for bb in fn.blocks:
            new_insts = []
            for inst in bb.instructions:
                si = inst.sync_info
                if si is not None and si.on_wait and len(si.on_wait) > max_waits:
                    waits = list(si.on_wait)
                    extra, keep = waits[:-max_waits], waits[-max_waits:]
                    while extra:
                        chunk, extra = extra[:max_waits], extra[max_waits:]
                        new_insts.append(mybir.InstNoOp(
                            name=nc.get_next_instruction_name(),
                            engine=inst.engine,
                            sync_info=mybir.SyncInfo(on_wait=chunk, on_update=[]),
                            bass_nofuse=True))
                    inst.sync_info = mybir.SyncInfo(
                        on_wait=keep, on_update=list(si.on_update))
                new_insts.append(inst)
            bb.instructions = new_insts


def build_nc():
    nc = bass.Bass("TRN2", target_bir_lowering=False, debug=False)

    xt_in = nc.declare_dram_parameter("xt", [128, 8, S], _bf, isOutput=False)
    xtq_in = nc.declare_dram_parameter("xtq", [128, 8, QS], _bf, isOutput=False)
    wk_in = nc.declare_dram_parameter("wk", [128, NPAIR * 128], _bf, isOutput=False)
    wq_in = nc.declare_dram_parameter("wq", [128, NPAIR * 128], _bf, isOutput=False)
    wv_in = nc.declare_dram_parameter("wv", [128, NPAIR * 128], _bf, isOutput=False)
    bk_in = nc.declare_dram_parameter("bk", [128, NPAIR], _f32, isOutput=False)
    bq_in = nc.declare_dram_parameter("bq", [128, NPAIR], _f32, isOutput=False)
    bv2_in = nc.declare_dram_parameter("bv2", [128, NPAIR], _f32, isOutput=False)
    sel2_in = nc.declare_dram_parameter("sel2", [128, NPAIR * 128], _bf, isOutput=False)
    wo_in = nc.declare_dram_parameter("wo", [128, 8, D], _bf, isOutput=False)
    bo_in = nc.declare_dram_parameter("bo", [1, D], _bf, isOutput=False)
    out_d = nc.declare_dram_parameter("out", [QS, D], _f32, isOutput=True)

    Exp = mybir.ActivationFunctionType.Exp
    Log = mybir.ActivationFunctionType.Ln

    with tile.TileContext(nc) as tc:
        with (
            tc.tile_pool(name="singles", bufs=1) as singles,
            tc.tile_pool(name="attn", bufs=3) as attn_pool,
            tc.tile_pool(name="sumt", bufs=2) as sumt_pool,
            tc.tile_pool(name="ysb", bufs=2) as y_pool,
        ):
            ones_bf = singles.tile([1, 128], _bf)
            nc.vector.memset(ones_bf[:], 1.0)
            warm_rhs = singles.tile([1, 512], _bf)
            nc.vector.memset(warm_rhs[:], 1.0)

            # --- input DMAs, priority-ordered: pair-0 projection inputs first
            wk_sb = singles.tile([128, NPAIR * 128], _bf)
            nc.sync.dma_start(wk_sb[:], wk_in[:])
            xt_sb = singles.tile([128, 8, S], _bf)
            nc.sync.dma_start(xt_sb[:, 0, :], xt_in[:, 0, :])
            wq_sb = singles.tile([128, NPAIR * 128], _bf)
            nc.sync.dma_start(wq_sb[:], wq_in[:])
            xtq_sb = singles.tile([128, 8, QS], _bf)
            nc.sync.dma_start(xtq_sb[:, 0:1, :], xtq_in[:, 0:1, :])
            bk_sb = singles.tile([128, NPAIR], _f32)
            nc.sync.dma_start(bk_sb[:], bk_in[:])
            bq_sb = singles.tile([128, NPAIR], _f32)
            nc.sync.dma_start(bq_sb[:], bq_in[:])
            wv_sb = singles.tile([128, NPAIR * 128], _bf)
            nc.sync.dma_start(wv_sb[:], wv_in[:])
            nc.sync.dma_start(xt_sb[:, 1, :], xt_in[:, 1, :])
            nc.sync.dma_start(xtq_sb[:, 1:8, :], xtq_in[:, 1:8, :])
            for p in range(2, NPAIR):
                nc.sync.dma_start(xt_sb[:, p, :], xt_in[:, p, :])
            bv2_sb = singles.tile([128, NPAIR], _f32)
            nc.sync.dma_start(bv2_sb[:], bv2_in[:])
            sel2_sb = singles.tile([128, NPAIR * 128], _bf)
            nc.sync.dma_start(sel2_sb[:], sel2_in[:])
            bo_sb = singles.tile([1, D], _bf)
            nc.sync.dma_start(bo_sb[:], bo_in[:])
            wo_sb = singles.tile([128, 8, D], _bf)
            nc.sync.dma_start(wo_sb[:], wo_in[:])

            kt_sb = singles.tile([128, NPAIR, S], _bf)
            qt_sb = singles.tile([128, NPAIR, QS], _bf)
            # V layout per (pair, keychunk): [V_A(64) | ones | V_B(64) | ones]
            v_sb = singles.tile([128, NPAIR, NKC, 130], _bf)

            otn = [singles.tile([128, QS], _bf, name=f"otn{p}") for p in range(NPAIR)]
            raw = [singles.tile([128, QS], _bf, name=f"raw{p}") for p in range(NPAIR)]
            sums_sb = singles.tile([98, QS], _bf)
            lnrow_sb = singles.tile([98, QS], _f32)
            recip_sb = singles.tile([98, QS], _bf)
            # the rank-66 bcast matmul reads all 66 partitions; zero-fill so
            # not-yet-written heads can't inject NaNs (0 x NaN = NaN on PE).
            # All big constant fills ride the otherwise-idle GpSimd engine.
            nc.gpsimd.memset(recip_sb[:], 0.0)
            for p in range(NPAIR):
                nc.gpsimd.memset(v_sb[:, p, :, :], 1.0)

            with (
                tc.tile_pool(name="pslot", bufs=3, space="PSUM") as slot_pool,
                tc.tile_pool(name="pot", bufs=1, space="PSUM") as ot_pool,
            ):
                def slot(nm):
                    return slot_pool.tile([128, 1024], _f32, tag="slot", name=nm)

                # PE warm-up: dummy matmuls (dep only on memsets) bring the HAM
                # clock gate toward K=8/8 while the first input DMAs land.
                wps = slot("warm")
                for i in range(5):
                    nc.tensor.matmul(wps[:, 0:512], ones_bf[:], warm_rhs[:],
                                     start=True, stop=True)

                def emit_kt(p, g):
                    ws = slice(p * 128, (p + 1) * 128)
                    ps = slot(f"kt{p}_{g}")
                    for i in range(2):
                        t0 = g * 1024 + i * 512
                        nc.tensor.matmul(
                            ps[:, i * 512:(i + 1) * 512],
                            wk_sb[:, ws],
                            xt_sb[:, p, t0:t0 + 512],
                            start=True, stop=True)
                    nc.vector.tensor_scalar_add(
                        kt_sb[:, p, g * 1024:(g + 1) * 1024], ps[:],
                        bk_sb[:, p:p + 1])

                def emit_qt(p):
                    ws = slice(p * 128, (p + 1) * 128)
                    psq = slot(f"qt{p}")
                    nc.tensor.matmul(psq[:, 0:QS], wq_sb[:, ws], xtq_sb[:, p, :],
                                     start=True, stop=True)
                    nc.vector.tensor_scalar_add(
                        qt_sb[:, p, :], psq[:, 0:QS], bq_sb[:, p:p + 1])

                # V proj group g: 4 token chunks, 4 N=128 matmuls into half a
                # slot, CAST out immediately (keeps the slot hold ~1 chunk).
                def emit_v(p, g):
                    ws = slice(p * 128, (p + 1) * 128)
                    psv = slot(f"v{p}_{g}")
                    psv4 = psv.rearrange("p (c e) -> p c e", e=128)
                    for i in range(4):
                        c = g * 4 + i
                        nc.tensor.matmul(
                            psv4[:, i, :],
                            xt_sb[:, p, c * 128:(c + 1) * 128],
                            wv_sb[:, ws],
                            start=True, stop=True)
                    dst = v_sb[:, p, g * 4:(g + 1) * 4, :].rearrange(
                        "p c (h e) -> p c h e", e=65)[:, :, :, 0:64]
                    src = psv[:, 0:512].rearrange(
                        "p (c h e) -> p c h e", h=2, e=64)
                    nc.vector.tensor_copy(dst, src)

                pots = {}

                def emit_scores_exp(p, c, ats):
                    pss = slot(f"pss{p}_{c}")
                    for a in range(2):
                        r = slice(64 * a, 64 * a + 64)
                        nc.tensor.matmul(
                            pss[:, a * QS:(a + 1) * QS],
                            kt_sb[r, p, c * 128:(c + 1) * 128],
                            qt_sb[r, p, :],
                            start=True, stop=True)
                    at = attn_pool.tile([128, 2 * QS], _bf, tag="at",
                                        name=f"at{p}_{c}")
                    nc.scalar.activation(at[:], pss[:], Exp, scale=SCALE)
                    ats.append(at)

                def emit_ot(p, c, ats):
                    at = ats.pop(0)
                    if c == 0:
                        pots[p] = (
                            ot_pool.tile([65, QS], _f32, tag="potA",
                                         name=f"potA{p}"),
                            ot_pool.tile([65, QS], _f32, tag="potB",
                                         name=f"potB{p}"),
                        )
                    pa, pb = pots[p]
                    nc.tensor.matmul(pa[:], v_sb[:, p, c, 0:65],
                                     at[:, 0:QS],
                                     start=(c == 0), stop=(c == NKC - 1))
                    nc.tensor.matmul(pb[:], v_sb[:, p, c, 65:130],
                                     at[:, QS:2 * QS],
                                     start=(c == 0), stop=(c == NKC - 1))

                def emit_stash(p):
                    pa, pb = pots.pop(p)
                    ra, rb = _sum_rows(p)
                    # One CAST per pot (dims + sums row together) frees the
                    # bank fast; raw[p] row 64 briefly holds A's sums until
                    # the B DMA overwrites it (DMA queue is FIFO).
                    nc.vector.tensor_copy(raw[p][0:65, :], pa[0:65, :])
                    braw = sumt_pool.tile([65, QS], _bf, tag="braw",
                                          name=f"braw{p}")
                    nc.vector.tensor_copy(braw[0:65, :], pb[0:65, :])
                    nc.sync.dma_start(sums_sb[ra:ra + 1, :], raw[p][64:65, :])
                    nc.sync.dma_start(sums_sb[rb:rb + 1, :], braw[64:65, :])
                    nc.sync.dma_start(raw[p][64:128, :], braw[0:64, :])

                def emit_recip(h0, h1):
                    # 1/sums via exp(-log(sums)): Ln and Exp share the
                    # natural_log_exp_and_others table set.
                    nc.scalar.activation(lnrow_sb[h0:h1, :], sums_sb[h0:h1, :],
                                         Log)
                    nc.scalar.activation(recip_sb[h0:h1, :], lnrow_sb[h0:h1, :],
                                         Exp, scale=-1.0)

                def emit_normalize(p):
                    nb = slot(f"nb{p}")
                    nc.tensor.matmul(nb[:, 0:QS],
                                     sel2_sb[0:98, p * 128:(p + 1) * 128],
                                     recip_sb[0:98, :], start=True, stop=True)
                    nc.vector.tensor_mul(otn[p][:, :], raw[p][:, :],
                                         nb[:, 0:QS])
                    nc.vector.tensor_scalar_add(otn[p][:, :], otn[p][:, :],
                                                bv2_sb[:, p:p + 1])

                # -------- interleave schedule: (p, c) -> list of thunks ------
                sched = {}

                def at_(p, c, fn, *args):
                    sched.setdefault((p, c), []).append((fn, args))

                # pair-1 projections ride early pair-0 chunk positions; its V
                # groups stack after pair-0's V (positions c0..c3).
                at_(0, 5, emit_kt, 1, 0)
                at_(0, 7, emit_kt, 1, 1)
                at_(0, 9, emit_qt, 1)
                at_(0, 10, emit_v, 1, 0)
                at_(0, 11, emit_v, 1, 1)
                at_(0, 12, emit_v, 1, 2)
                at_(0, 13, emit_v, 1, 3)
                for p in range(1, NPAIR - 1):
                    pn = p + 1
                    at_(p, 3, emit_kt, pn, 0)
                    at_(p, 5, emit_kt, pn, 1)
                    at_(p, 7, emit_qt, pn)
                    at_(p, 8, emit_v, pn, 0)
                    at_(p, 10, emit_v, pn, 1)
                    at_(p, 12, emit_v, pn, 2)
                    at_(p, 14, emit_v, pn, 3)
                for p in range(1, NPAIR):
                    at_(p, 1, emit_stash, p - 1)
                at_(4, 6, emit_recip, 0, 8)
                at_(6, 4, emit_recip, 32, 36)
                at_(7, 4, emit_recip, 96, 98)
                at_(4, 9, emit_normalize, 0)
                at_(4, 11, emit_normalize, 1)
                at_(4, 13, emit_normalize, 2)
                at_(5, 2, emit_normalize, 3)
                at_(6, 6, emit_normalize, 4)
                at_(6, 9, emit_normalize, 5)
                at_(7, 6, emit_normalize, 6)

                # -------- projections for pair 0 up front (V rides the first
                # chain positions so scores/exp start sooner) -----------------
                emit_kt(0, 0)
                emit_qt(0)
                emit_kt(0, 1)
                for g in range(4):
                    at_(0, g, emit_v, 0, g)

                # -------- global chunk pipeline ------------------------------
                ats = []
                for g in range(NPAIR * NKC + 2):
                    if g < NPAIR * NKC:
                        p, c = divmod(g, NKC)
                        emit_scores_exp(p, c, ats)
                    if g >= 2:
                        po, co = divmod(g - 2, NKC)
                        emit_ot(po, co, ats)
                    if g < NPAIR * NKC:
                        for fn, args in sched.get((p, c), ()):
                            fn(*args)

                # -------- tail: last pair epilogue + output projection -------
                emit_stash(NPAIR - 1)
                emit_recip(64, 66)

                ybufs = {}

                def emit_y_head(j):
                    # one LDWEIGHTS per otn[k] j-slice serves both dout halves
                    py = slot(f"y{j}")
                    ybufs[j] = py
                    for k in range(NPAIR - 1):
                        for dh in range(2):
                            nc.tensor.matmul(
                                py[:, dh * 512:(dh + 1) * 512],
                                otn[k][:, j * 128:(j + 1) * 128],
                                wo_sb[:, k, dh * 512:(dh + 1) * 512],
                                start=(k == 0), stop=False)

                def emit_y_fin(j):
                    py = ybufs.pop(j)
                    k = NPAIR - 1
                    for dh in range(2):
                        nc.tensor.matmul(py[:, dh * 512:(dh + 1) * 512],
                                         otn[k][:, j * 128:(j + 1) * 128],
                                         wo_sb[:, k, dh * 512:(dh + 1) * 512],
                                         start=False, stop=False)
                    for dh in range(2):
                        nc.tensor.matmul(py[:, dh * 512:(dh + 1) * 512],
                                         ones_bf[0:1, :],
                                         bo_sb[0:1, dh * 512:(dh + 1) * 512],
                                         start=False, stop=True)
                    ysb = y_pool.tile([128, 1024], _f32, tag="ysb")
                    nc.vector.tensor_copy(ysb[:], py[:])
                    nc.sync.dma_start(out_d[j * 128:(j + 1) * 128, :], ysb[:])

                # y heads for j=0,1 run on PE while the pair-7 epilogue
                # finishes on DVE/ACT (keeps HAM warm); nb7 between them.
                emit_y_head(0)
                emit_normalize(NPAIR - 1)
                emit_y_head(1)
                emit_y_fin(0)
                emit_y_head(2)
                emit_y_fin(1)
                emit_y_head(3)
                emit_y_fin(2)
                emit_y_fin(3)

    _split_excess_waits(nc, 1)
    return nc


def _blockdiag_pack(w):
    """[H, HD, HD] -> [128, NPAIR*128] blockdiagonal per pair, k-major."""
    out = np.zeros((128, NPAIR * 128), np.float32)
    for p in range(NPAIR):
        out[0:64, p * 128 + 0:p * 128 + 64] = w[2 * p]
        out[64:128, p * 128 + 64:p * 128 + 128] = w[2 * p + 1]
    return out.astype(BF16)


def _bias_pack(b):
    """[H, HD] -> [128, NPAIR] (pair bias along partitions)."""
    out = np.zeros((128, NPAIR), np.float32)
    for p in range(NPAIR):
        out[0:64, p] = b[2 * p]
        out[64:128, p] = b[2 * p + 1]
    return out


def _sel2_pack():
    """[128, NPAIR*128]: pair p block has row _sum_rows(p)[0] = [ones64|0],
    row _sum_rows(p)[1] = [0|ones64] so one rank-66 matmul broadcasts both
    heads' recip rows to the pair's 128 output partitions."""
    out = np.zeros((128, NPAIR * 128), np.float32)
    for p in range(NPAIR):
        ra, rb = _sum_rows(p)
        out[ra, p * 128:p * 128 + 64] = 1.0
        out[rb, p * 128 + 64:p * 128 + 128] = 1.0
    return out.astype(BF16)


def prepare_inputs(X, Wq, bq, Wk, bk, Wv, bv, Wo, bo):
    """Host-side shard + pack. Returns in_maps (one dict per core)."""
    X = np.asarray(X, np.float32)
    common = {
        "wk": _blockdiag_pack(np.asarray(Wk, np.float32)),
        "wq": _blockdiag_pack(np.asarray(Wq, np.float32)),
        "wv": _blockdiag_pack(np.asarray(Wv, np.float32)),
        "bk": _bias_pack(np.asarray(bk, np.float32)),
        "bq": _bias_pack(np.asarray(bq, np.float32)),
        "bv2": _bias_pack(np.asarray(bv, np.float32)),
        "sel2": _sel2_pack(),
        "wo": np.ascontiguousarray(
            np.asarray(Wo, np.float32).reshape(8, 128, D).transpose(1, 0, 2)
        ).astype(BF16),
        "bo": np.asarray(bo, np.float32).reshape(1, D).astype(BF16),
    }
    xts = []
    for b in range(B):
        xt = np.ascontiguousarray(X[b].T)                   # [D, S]
        xts.append(np.ascontiguousarray(
            xt.reshape(8, 128, S).transpose(1, 0, 2)).astype(BF16))
    in_maps = []
    for c in range(NCORES):
        b = c // (NCORES // B)
        q0 = (c % (NCORES // B)) * QS
        m = dict(common)
        m["xt"] = xts[b]
        m["xtq"] = np.ascontiguousarray(xts[b][:, :, q0:q0 + QS])
        in_maps.append(m)
    return in_maps


_NC_CACHE = None


def _get_nc():
    global _NC_CACHE
    if _NC_CACHE is None:
        _NC_CACHE = build_nc()
    return _NC_CACHE


def kernel(X, Wq, bq, Wk, bk, Wv, bv, Wo, bo):
    nc = _get_nc()
    in_maps = prepare_inputs(X, Wq, bq, Wk, bk, Wv, bv, Wo, bo)
    res = run_bass_kernel_spmd(nc, in_maps, core_ids=list(range(NCORES)))
    out = np.empty((B, S, D), np.float32)
    for c in range(NCORES):
        b = c // (NCORES // B)
        q0 = (c % (NCORES // B)) * QS
        out[b, q0:q0 + QS, :] = res.results[c]["out"]
    return out
